# revision 14
# baseline (speedup 1.0000x reference)
"""DiT block kernel for 8 Trainium2 NeuronCores — fp8 DoubleRow edition.

Sharding: data-parallel over batch (B=8 -> one batch element per core).

Vs the bf16 baseline:
  - QKV / attn@V / proj run as fp8(e4m3) DoubleRow matmuls: 2 contraction
    planes per instruction at 0.5 cyc/row (2x bf16 PE rate). Weights are
    per-column absmax-quantized host-side; dequant scales fold into
    eviction scalars / gate broadcast tiles.
  - scores: fp8 operands, bf16-rate matmuls, 512-wide moving dim.
  - softmax denominator via a DR ones-matmul whose output arrives
    pre-broadcast [64, 512]; normalize+quantize of attn@V is ONE fused
    scalar_tensor_tensor (mult, divide) per head per qchunk.
  - LN: token-major norm, PE transpose, modulate + quantize fused into a
    single tensor_scalar at PSUM eviction (per-feature scale/shift are
    per-partition scalars after the transpose).
  - fc1 PSUM evicts straight through ACT gelu (bias as per-partition AP).
  - fc1/fc2/ada stay bf16 (fp8 there costs too much accuracy), but with
    512-wide moving operands.
"""

import sys

sys.path.insert(0, "/opt/trn_rl_repo")

import numpy as np
import ml_dtypes

import concourse.bacc as bacc
import concourse.tile as tile
from concourse import mybir
from concourse import bass_utils
from concourse.masks import make_identity

F32 = mybir.dt.float32
BF16 = mybir.dt.bfloat16
FP8 = mybir.dt.float8e4
BF = ml_dtypes.bfloat16
F8NP = ml_dtypes.float8_e4m3
OP = mybir.AluOpType
AF = mybir.ActivationFunctionType
DRM = mybir.MatmulPerfMode.DoubleRow

B = 8
L = 1024
D = 1024
H = 16
HD = 64
FF = 4096
LN_EPS = 1e-5
P = 128
TT = L // P          # 8 token tiles
KT = D // P          # 8 feature tiles
FT = FF // P         # 32 ff tiles
QC = 512             # query chunk
NQC = L // QC        # 2

S_H = 16.0           # ln1 modulated-output fp8 scale
S_V = 32.0           # v fp8 scale
S_QK = 32.0          # q/k fp8 scale
S_A = 32.0           # attn-out fp8 scale
ESC = 0.125 / (S_QK * S_QK)

_CACHE = {}


def _build():
    nc = bacc.Bacc(None, target_bir_lowering=False)
    names = {}
    with tile.TileContext(nc) as tc, \
            tc.tile_pool(name="dram", bufs=1, space="DRAM") as dram, \
            tc.tile_pool(name="per", bufs=1) as per:

        # ---------------- DRAM inputs (host-prepped layouts) -------------
        x_d = dram.tile([P, TT, D], F32, kind="ExternalInput", name="x")
        c_d = dram.tile([P, KT], F32, kind="ExternalInput", name="c")
        lnT_d = dram.tile([P, 4 * KT], F32, kind="ExternalInput", name="lnT")
        adaw_d = dram.tile([P, KT, 6 * D], BF16, kind="ExternalInput",
                           name="ada_w")
        adab_d = dram.tile([1, 6 * D], BF16, kind="ExternalInput",
                           name="ada_b")
        qkvw_d = dram.tile([P, KT, 3 * D], FP8, kind="ExternalInput",
                           name="qkv_w")
        dqqk_d = dram.tile([64, 32], F32, kind="ExternalInput", name="dqqk")
        bqk_d = dram.tile([64, 32], F32, kind="ExternalInput", name="bqk")
        vbs_d = dram.tile([1, D], BF16, kind="ExternalInput", name="vbs")
        dqv_d = dram.tile([1, D], BF16, kind="ExternalInput", name="dqv")
        projw_d = dram.tile([P, KT, D], FP8, kind="ExternalInput",
                            name="proj_w")
        pbs_d = dram.tile([1, D], BF16, kind="ExternalInput", name="pbs")
        dqp_d = dram.tile([1, D], F32, kind="ExternalInput", name="dqp")
        fc1w_d = dram.tile([P, KT, FF], BF16, kind="ExternalInput",
                           name="fc1_w")
        bf1_d = dram.tile([P, FT], F32, kind="ExternalInput", name="bf1")
        fc2w_d = dram.tile([P, FT, D], BF16, kind="ExternalInput",
                           name="fc2_w")
        f2bs_d = dram.tile([1, D], BF16, kind="ExternalInput", name="f2bs")
        out_d = dram.tile([L, D], F32, kind="ExternalOutput", name="out")
        for t, n in [(x_d, "x"), (c_d, "c"), (lnT_d, "lnT"),
                     (adaw_d, "ada_w"), (adab_d, "ada_b"),
                     (qkvw_d, "qkv_w"), (dqqk_d, "dqqk"), (bqk_d, "bqk"),
                     (vbs_d, "vbs"), (dqv_d, "dqv"), (projw_d, "proj_w"),
                     (pbs_d, "pbs"), (dqp_d, "dqp"), (fc1w_d, "fc1_w"),
                     (bf1_d, "bf1"), (fc2w_d, "fc2_w"), (f2bs_d, "f2bs"),
                     (out_d, "out")]:
            names[n] = t.name
        out_v = out_d[:].rearrange("(t p) d -> p t d", p=P)

        # ---------------- persistent tiles ------------------------------
        x_sb = per.tile([P, TT, D], F32)
        for t in range(TT):
            nc.sync.dma_start(x_sb[:, t, :], x_d[:, t, :])
        ident = per.tile([P, P], BF16)
        make_identity(nc, ident[:])
        ones8 = per.tile([P, 2, HD], FP8)      # DR denominator lhsT
        nc.vector.memset(ones8[:], 1.0)
        onesrow = per.tile([1, P], BF16)       # bias-broadcast lhsT/moving
        nc.vector.memset(onesrow[:], 1.0)
        eps_sb = per.tile([P, 1], F32)
        nc.vector.memset(eps_sb[:], LN_EPS)

        lnT = per.tile([P, 4 * KT], F32)
        nc.sync.dma_start(lnT[:], lnT_d[:])
        dqqk = per.tile([64, 32], F32)
        nc.sync.dma_start(dqqk[:], dqqk_d[:])
        bqk = per.tile([64, 32], F32)
        nc.sync.dma_start(bqk[:], bqk_d[:])
        vbs = per.tile([1, D], BF16)
        nc.sync.dma_start(vbs[:], vbs_d[:])
        dqv_row = per.tile([1, D], BF16)
        nc.sync.dma_start(dqv_row[:], dqv_d[:])
        pbs = per.tile([1, D], BF16)
        nc.sync.dma_start(pbs[:], pbs_d[:])
        dqp_row = per.tile([1, D], F32)
        nc.sync.dma_start(dqp_row[:], dqp_d[:])
        bf1 = per.tile([P, FT], F32)
        nc.sync.dma_start(bf1[:], bf1_d[:])
        f2bs = per.tile([1, D], BF16)
        nc.sync.dma_start(f2bs[:], f2bs_d[:])
        adab = per.tile([1, 6 * D], BF16)
        nc.sync.dma_start(adab[:], adab_d[:])
        projw = per.tile([P, KT, D], FP8)
        for k in range(KT):
            nc.sync.dma_start(projw[:, k, :], projw_d[:, k, :])

        eff1s = per.tile([P, KT], F32)
        eff1h = per.tile([P, KT], F32)
        eff2s = per.tile([P, KT], F32)
        eff2h = per.tile([P, KT], F32)
        g1bc = per.tile([P, D], BF16)
        g2bc = per.tile([P, D], BF16)
        dqvbc = per.tile([P, D], BF16)
        nc.gpsimd.partition_broadcast(dqvbc[:], dqv_row[:])

        h1T = per.tile([P, KT, L], FP8)
        q8 = per.tile([P, KT, L], FP8)
        k8 = per.tile([P, KT, L], FP8)
        v8 = per.tile([P, TT, H, HD], FP8)
        geluT = per.tile([P, FT, QC], BF16)
        h2T = per.tile([P, KT, QC], BF16)

        _work_cm = tc.tile_pool(name="work", bufs=2)
        work = _work_cm.__enter__()

        def ln_stats(x_aps):
            n = len(x_aps)
            mv = work.tile([P, n, 2], F32, tag=f"mv{n}", name=f"mv{n}")
            for i, x_ap in enumerate(x_aps):
                stats = work.tile([P, 2, 6], F32, tag="stats")
                for sg in range(2):
                    nc.vector.bn_stats(stats[:, sg, :],
                                       x_ap[:, sg * 512:(sg + 1) * 512])
                nc.vector.bn_aggr(mv[:, i, :], stats[:])
            rstd = work.tile([P, n], F32, tag=f"rstd{n}", name=f"rstd{n}")
            nc.scalar.activation(rstd[:], mv[:, :, 1], AF.Sqrt, bias=eps_sb[:])
            nc.vector.reciprocal(rstd[:], rstd[:])
            return mv, rstd

        mv1, rstd1 = ln_stats([x_sb[:, t, :] for t in range(TT)])

        # ---------------- ada (bf16, 512-wide moving weights) -----------
        with nc.named_scope("ada"), \
                tc.tile_pool(name="ada_sb", bufs=1) as asb, \
                tc.tile_pool(name="ada_stream", bufs=2) as ast, \
                tc.tile_pool(name="ps_ada", bufs=2, space="PSUM") as psa, \
                tc.tile_pool(name="ps_t6", bufs=1, space="PSUM") as pst6:
            c_sb = asb.tile([P, KT], F32)
            nc.sync.dma_start(c_sb[:], c_d[:])
            silu_b = asb.tile([P, KT], BF16)
            nc.scalar.activation(silu_b[:], c_sb[:], AF.Silu)
            ssgb = asb.tile([1, 6 * D], BF16, name="ssgb")
            for nch in range(12):
                aw = ast.tile([P, KT, 512], BF16, tag="aw")
                nc.sync.dma_start(aw[:],
                                  adaw_d[:, :, nch * 512:(nch + 1) * 512])
                pa = psa.tile([1, 512], F32, tag="ada")
                nc.tensor.matmul(pa[:], onesrow[:, 0:1],
                                 adab[:, nch * 512:(nch + 1) * 512],
                                 start=True, stop=False)
                for k in range(KT):
                    nc.tensor.matmul(pa[:], silu_b[:, k:k + 1], aw[:, k, :],
                                     start=False, stop=(k == KT - 1),
                                     skip_group_check=True)
                nc.vector.tensor_copy(
                    ssgb[:, nch * 512:(nch + 1) * 512], pa[:])
            # shift/scale rows -> feature-major cols ssgT[p, j, v'];
            # v' order: sh1, sc1, sh2, sc2 (= ssg vectors 0, 1, 3, 4)
            # stride-2 columns keep each bf16 PSUM write 4-byte aligned
            pt6 = pst6.tile([P, 32, 2], BF16)
            for vi, v in enumerate((0, 1, 3, 4)):
                for ch in range(KT):
                    i = ch * 4 + vi
                    nc.tensor.transpose(
                        pt6[:, i, 0:1],
                        ssgb[0:1, v * D + ch * P:v * D + (ch + 1) * P],
                        ident[0:1, 0:1])
            ssgT = asb.tile([P, KT, 4], F32, name="ssgT")
            nc.vector.tensor_copy(ssgT[:], pt6[:, :, 0])
            t0 = asb.tile([P, KT], F32, name="t0")
            t1 = asb.tile([P, KT], F32, name="t1")
            # eff1s = (1+sc1)*ln1w*S_H ; eff1h = (1+sc1)*ln1b*S_H + sh1*S_H
            nc.vector.tensor_scalar_add(t0[:], ssgT[:, :, 1], 1.0)
            nc.vector.tensor_mul(eff1s[:], t0[:], lnT[:, 0:KT])
            nc.vector.tensor_mul(t1[:], t0[:], lnT[:, KT:2 * KT])
            nc.vector.tensor_scalar(out=eff1h[:], in0=ssgT[:, :, 0],
                                    scalar1=S_H, scalar2=1.0,
                                    op0=OP.mult, op1=OP.mult)
            nc.vector.tensor_add(eff1h[:], eff1h[:], t1[:])
            # eff2s = (1+sc2)*ln2w ; eff2h = (1+sc2)*ln2b + sh2  (bf16 mlp)
            nc.vector.tensor_scalar_add(t0[:], ssgT[:, :, 3], 1.0)
            nc.vector.tensor_mul(eff2s[:], t0[:], lnT[:, 2 * KT:3 * KT])
            nc.vector.tensor_mul(t1[:], t0[:], lnT[:, 3 * KT:4 * KT])
            nc.vector.tensor_add(eff2h[:], t1[:], ssgT[:, :, 2])
            # gates: g1 row folds proj dequant; g2 raw
            g1row = asb.tile([1, D], BF16, name="g1row")
            g2row = asb.tile([1, D], BF16, name="g2row")
            nc.vector.tensor_mul(g1row[:], ssgb[:, 2 * D:3 * D], dqp_row[:])
            nc.gpsimd.partition_broadcast(g1bc[:], g1row[:])
            nc.gpsimd.partition_broadcast(g2bc[:], ssgb[:, 5 * D:6 * D])

        # ---------------- LN -> hT (transpose + fused modulate) ---------
        def build_hT(dst, eff_s, eff_h, mv, rstd, idx0, tg, toff):
            with tc.tile_pool(name="ps_tr", bufs=2, space="PSUM") as pstr:
                norms = []
                for s in range(4):
                    t = toff + s
                    nrm = work.tile([P, D], BF16, tag="nrm", bufs=4)
                    i = idx0 + s
                    nc.vector.tensor_scalar(out=nrm[:], in0=x_sb[:, t, :],
                                            scalar1=mv[:, i, 0:1],
                                            scalar2=rstd[:, i:i + 1],
                                            op0=OP.subtract, op1=OP.mult)
                    norms.append(nrm)
                for k in range(KT):
                    pt = pstr.tile([P, 4 * P], BF16, tag="tr")
                    for s in range(4):
                        nc.tensor.transpose(pt[:, s * P:(s + 1) * P],
                                            norms[s][:, k * P:(k + 1) * P],
                                            ident[:])
                    nc.vector.tensor_scalar(
                        out=dst[:, k, tg * 512:(tg + 1) * 512], in0=pt[:],
                        scalar1=eff_s[:, k:k + 1], scalar2=eff_h[:, k:k + 1],
                        op0=OP.mult, op1=OP.add)

        for tg in range(2):
            build_hT(h1T, eff1s, eff1h, mv1, rstd1, tg * 4, tg, tg * 4)

        # ---------------- QKV (fp8 DoubleRow) ---------------------------
        with nc.named_scope("qkv"), \
                tc.tile_pool(name="qkv_stream", bufs=3) as qst, \
                tc.tile_pool(name="ps_qkv", bufs=3, space="PSUM") as psq:
            for qk in range(2):          # 0 = K, 1 = Q
                fbase = D if qk == 0 else 0
                dst = k8 if qk == 0 else q8
                for jj in range(KT):
                    wj = qst.tile([P, KT, P], FP8, tag="wj")
                    nc.sync.dma_start(
                        wj[:],
                        qkvw_d[:, :, fbase + jj * P:fbase + (jj + 1) * P])
                    for hh in range(2):
                        ci = (fbase + jj * P) // 64 + hh
                        for tg in range(2):
                            pq = psq.tile([64, 512], F32, tag="pqk")
                            for tq in range(2):
                                for kp in range(4):
                                    nc.tensor.matmul(
                                        pq[:, tq * 256:(tq + 1) * 256],
                                        wj[:, 2 * kp:2 * kp + 2,
                                           hh * 64:(hh + 1) * 64],
                                        h1T[:, 2 * kp:2 * kp + 2,
                                            tg * 512 + tq * 256:
                                            tg * 512 + (tq + 1) * 256],
                                        start=(kp == 0), stop=(kp == 3),
                                        perf_mode=DRM)
                            nc.vector.tensor_scalar(
                                out=dst[hh * 64:(hh + 1) * 64, jj,
                                        tg * 512:(tg + 1) * 512],
                                in0=pq[:], scalar1=dqqk[:, ci:ci + 1],
                                scalar2=bqk[:, ci:ci + 1],
                                op0=OP.mult, op1=OP.add)
            # V: token-major out; weights are the moving operand
            for fh in range(2):
                wv = qst.tile([P, KT, 512], FP8, tag="wv")
                nc.sync.dma_start(wv[:],
                                  qkvw_d[:, :, 2 * D + fh * 512:
                                         2 * D + (fh + 1) * 512])
                for tc_ in range(16):
                    tt, tp = tc_ // 2, (tc_ % 2) * 64
                    pv = psq.tile([64, 512], F32, tag="pqk")
                    nc.tensor.matmul(pv[:], onesrow[:, 0:64],
                                     vbs[:, fh * 512:(fh + 1) * 512],
                                     start=True, stop=False)
                    for fq in range(2):
                        for kp in range(4):
                            nc.tensor.matmul(
                                pv[:, fq * 256:(fq + 1) * 256],
                                h1T[:, 2 * kp:2 * kp + 2,
                                    tc_ * 64:(tc_ + 1) * 64],
                                wv[:, 2 * kp:2 * kp + 2,
                                   fq * 256:(fq + 1) * 256],
                                start=False, stop=(fq == 1 and kp == 3),
                                perf_mode=DRM, skip_group_check=True)
                    nc.vector.tensor_tensor(
                        out=v8[tp:tp + 64, tt, fh * 8:(fh + 1) * 8, :],
                        in0=pv[:],
                        in1=dqvbc[0:64, fh * 512:(fh + 1) * 512],
                        op=OP.mult)

        # ---------------- attention + MLP, pipelined --------------------
        with tc.tile_pool(name="attn", bufs=2) as ap, \
                tc.tile_pool(name="aTp", bufs=2) as aTp, \
                tc.tile_pool(name="fc_stream", bufs=3) as fs, \
                tc.tile_pool(name="ps_s", bufs=2, space="PSUM") as pss, \
                tc.tile_pool(name="ps_av", bufs=1, space="PSUM") as psav, \
                tc.tile_pool(name="ps_m", bufs=2, space="PSUM") as psm:

            def scores_exp(qc, j):
                q0 = qc * QC
                attA = ap.tile([P, TT, QC], FP8, tag="attA")
                attB = ap.tile([P, TT, QC], FP8, tag="attB")
                for m in range(TT):
                    psA = pss.tile([P, QC], F32, tag="sc", name="psA")
                    psB = pss.tile([P, QC], F32, tag="sc", name="psB")
                    nc.tensor.matmul(psA[:], k8[0:HD, j, m * P:(m + 1) * P],
                                     q8[0:HD, j, q0:q0 + QC],
                                     start=True, stop=True)
                    nc.tensor.matmul(psB[:], k8[HD:P, j, m * P:(m + 1) * P],
                                     q8[HD:P, j, q0:q0 + QC],
                                     start=True, stop=True,
                                     tile_position=(HD, 0))
                    nc.scalar.activation(attA[:, m, :], psA[:], AF.Exp,
                                         scale=ESC)
                    nc.scalar.activation(attB[:, m, :], psB[:], AF.Exp,
                                         scale=ESC)
                return attA, attB

            def av_norm(j, attA, attB, aT8):
                for hh, att in ((0, attA), (1, attB)):
                    h = 2 * j + hh
                    pu = psav.tile([64, QC], F32, tag="pu")
                    den = psav.tile([64, QC], F32, tag="den")
                    for qh in range(2):
                        for g in range(4):
                            nc.tensor.matmul(
                                pu[:, qh * 256:(qh + 1) * 256],
                                v8[:, 2 * g:2 * g + 2, h, :],
                                att[:, 2 * g:2 * g + 2,
                                    qh * 256:(qh + 1) * 256],
                                start=(g == 0), stop=(g == 3),
                                perf_mode=DRM)
                    for qh in range(2):
                        for g in range(4):
                            nc.tensor.matmul(
                                den[:, qh * 256:(qh + 1) * 256],
                                ones8[:],
                                att[:, 2 * g:2 * g + 2,
                                    qh * 256:(qh + 1) * 256],
                                start=(g == 0), stop=(g == 3),
                                perf_mode=DRM)
                    rec = work.tile([64, QC], BF16, tag="rec")
                    with nc.allow_low_precision(reason="softmax denom recip"):
                        nc.vector.reciprocal(rec[:], den[:])
                    nc.vector.scalar_tensor_tensor(
                        out=aT8[hh * 64:(hh + 1) * 64, j, :],
                        in0=pu[:], scalar=S_A / S_V, in1=rec[:],
                        op0=OP.mult, op1=OP.mult)

            def proj_block(qc, aT8, s):
                t_global = qc * 4 + s // 2
                xp = (s % 2) * 64
                for fh in range(2):
                    pp = psm.tile([P, 512], F32, tag="mm", name="pp")
                    nc.tensor.matmul(pp[0:64, :], onesrow[:, 0:64],
                                     pbs[:, fh * 512:(fh + 1) * 512],
                                     start=True, stop=False)
                    for jp in range(4):
                        for fq in range(2):
                            nc.tensor.matmul(
                                pp[0:64, fq * 256:(fq + 1) * 256],
                                aT8[:, 2 * jp:2 * jp + 2,
                                    s * 64:(s + 1) * 64],
                                projw[:, 2 * jp:2 * jp + 2,
                                      fh * 512 + fq * 256:
                                      fh * 512 + (fq + 1) * 256],
                                start=False, stop=(jp == 3 and fq == 1),
                                perf_mode=DRM, skip_group_check=True)
                    tmp = work.tile([P, 512], BF16, tag="ptmp", bufs=1)
                    nc.vector.tensor_tensor(
                        out=tmp[xp:xp + 64, :], in0=pp[0:64, :],
                        in1=g1bc[xp:xp + 64, fh * 512:(fh + 1) * 512],
                        op=OP.mult)
                    nc.vector.tensor_add(
                        x_sb[xp:xp + 64, t_global, fh * 512:(fh + 1) * 512],
                        x_sb[xp:xp + 64, t_global, fh * 512:(fh + 1) * 512],
                        tmp[xp:xp + 64, :])

            def mlp_blocks(qc):
                """fc1+gelu then fc2 for tokens [qc*QC, (qc+1)*QC)."""
                for fcg in range(8):           # fc1: 8 groups of 4 chunks
                    def fc1_block(fcg=fcg, qc=qc):
                        for fi in range(4):
                            fc = fcg * 4 + fi
                            w1 = fs.tile([P, KT, P], BF16, tag="w1", bufs=2)
                            nc.sync.dma_start(
                                w1[:], fc1w_d[:, :, fc * P:(fc + 1) * P])
                            pf = psm.tile([P, 512], F32, tag="mm", name="pf")
                            for k in range(KT):
                                nc.tensor.matmul(
                                    pf[:], w1[:, k, :],
                                    h2T[:, k, :],
                                    start=(k == 0), stop=(k == KT - 1))
                            nc.scalar.activation(geluT[:, fc, :], pf[:],
                                                 AF.Gelu_apprx_tanh,
                                                 bias=bf1[:, fc:fc + 1])
                    yield fc1_block
                for s in range(4):             # fc2: 4 token tiles of 128
                    def fc2_block(s=s, qc=qc):
                        t_global = qc * 4 + s
                        ot = work.tile([P, D], F32, tag="ot")
                        for fp in range(2):
                            pf2 = psm.tile([P, 512], F32, tag="mm",
                                           name="pf2")
                            nc.tensor.matmul(
                                pf2[:], onesrow[:, 0:P],
                                f2bs[:, fp * 512:(fp + 1) * 512],
                                start=True, stop=False)
                            for sub in range(2):
                                for fg in range(2):
                                    w2 = fs.tile([P, FT // 2, 256], BF16,
                                                 tag="w2", bufs=2)
                                    nc.sync.dma_start(
                                        w2[:],
                                        fc2w_d[:, fg * 16:(fg + 1) * 16,
                                               fp * 512 + sub * 256:
                                               fp * 512 + (sub + 1) * 256])
                                    for ft in range(FT // 2):
                                        nc.tensor.matmul(
                                            pf2[:, sub * 256:(sub + 1) * 256],
                                            geluT[:, fg * 16 + ft,
                                                  s * P:(s + 1) * P],
                                            w2[:, ft, :],
                                            start=False,
                                            stop=(sub == 1 and fg == 1
                                                  and ft == FT // 2 - 1),
                                            skip_group_check=True)
                            tmp = work.tile([P, 512], BF16, tag="ftmp", bufs=1)
                            nc.vector.tensor_tensor(
                                out=tmp[:], in0=pf2[:],
                                in1=g2bc[:, fp * 512:(fp + 1) * 512],
                                op=OP.mult)
                            nc.vector.tensor_add(
                                ot[:, fp * 512:(fp + 1) * 512],
                                x_sb[:, t_global, fp * 512:(fp + 1) * 512],
                                tmp[:])
                        nc.sync.dma_start(out_v[:, t_global, :], ot[:])
                    yield fc2_block

            pending = []
            pend_i = 0
            for qc in range(NQC):
                with nc.named_scope(f"attn{qc}"):
                    aT8 = aTp.tile([P, KT, QC], FP8, tag="aT8",
                                   name=f"aT8_{qc}")
                    atts = []
                    for j in range(KT):
                        atts.append(scores_exp(qc, j))
                        if j >= 1:
                            av_norm(j - 1, *atts[j - 1], aT8)
                        # drain pending mlp blocks of the previous chunk
                        for _ in range(2):
                            if pend_i < len(pending):
                                pending[pend_i]()
                                pend_i += 1
                    av_norm(KT - 1, *atts[KT - 1], aT8)
                    while pend_i < len(pending):
                        pending[pend_i]()
                        pend_i += 1
                with nc.named_scope(f"proj{qc}"):
                    for s in range(8):
                        proj_block(qc, aT8, s)
                with nc.named_scope(f"ln2_{qc}"):
                    mv2, rstd2 = ln_stats(
                        [x_sb[:, qc * 4 + s, :] for s in range(4)])
                    build_hT(h2T, eff2s, eff2h, mv2, rstd2, 0, 0, qc * 4)
                pending = list(mlp_blocks(qc))
                pend_i = 0
            with nc.named_scope("mlp_tail"):
                while pend_i < len(pending):
                    pending[pend_i]()
                    pend_i += 1

        _work_cm.__exit__(None, None, None)

    nc.compile()
    return nc, names


def _get_compiled():
    if "nc" not in _CACHE:
        _CACHE["nc"], _CACHE["names"] = _build()
    return _CACHE["nc"], _CACHE["names"]


def _q8col(w):
    """Per-column absmax quantize to TRN e4m3. Returns (w8, scale_col)."""
    w = np.asarray(w, np.float32)
    am = np.abs(w).max(axis=0, keepdims=True)
    s = np.where(am > 0, 224.0 / np.maximum(am, 1e-30), 1.0)
    w8 = np.clip(w * s, -240, 240).astype(F8NP)
    return w8, s[0]


def _pmajor(w):
    w = np.asarray(w)
    kp, n = w.shape
    return np.ascontiguousarray(w.reshape(kp // P, P, n).transpose(1, 0, 2))


def _prep_maps(names, x, c, ln1_w, ln1_b, ln2_w, ln2_b, ada_w, ada_b,
               qkv_w, qkv_b, proj_w, proj_b, fc1_w, fc1_b, fc2_w, fc2_b):
    x = np.asarray(x, np.float32)
    c = np.asarray(c, np.float32)

    qkv8, s_qkv = _q8col(qkv_w)
    proj8, s_proj = _q8col(proj_w)
    qkv_b = np.asarray(qkv_b, np.float32)

    # lnT columns: [ln1w*S_H | ln1b*S_H | ln2w | ln2b], feature-major
    def tcols(v, s=1.0):
        return (np.asarray(v, np.float32) * s).reshape(KT, P).T
    lnT = np.concatenate([tcols(ln1_w, S_H), tcols(ln1_b, S_H),
                          tcols(ln2_w), tcols(ln2_b)], axis=1)

    dqqk = np.ascontiguousarray(
        (S_QK / (S_H * s_qkv[0:2 * D])).reshape(32, 64).T)
    bqk = np.ascontiguousarray(
        (qkv_b[0:2 * D] * S_QK).reshape(32, 64).T)
    vbs = (qkv_b[2 * D:] * S_H * s_qkv[2 * D:]).astype(BF).reshape(1, D)
    dqv = (S_V / (S_H * s_qkv[2 * D:])).astype(BF).reshape(1, D)
    pbs = (np.asarray(proj_b, np.float32) * S_A * s_proj).astype(BF)
    dqp = (1.0 / (S_A * s_proj)).astype(np.float32).reshape(1, D)

    common = {
        names["lnT"]: np.ascontiguousarray(lnT, np.float32),
        names["ada_w"]: _pmajor(ada_w).astype(BF),
        names["ada_b"]: np.asarray(ada_b).astype(BF).reshape(1, -1),
        names["qkv_w"]: _pmajor(qkv8),
        names["dqqk"]: dqqk.astype(np.float32),
        names["bqk"]: bqk.astype(np.float32),
        names["vbs"]: vbs,
        names["dqv"]: dqv,
        names["proj_w"]: _pmajor(proj8),
        names["pbs"]: pbs.reshape(1, D),
        names["dqp"]: dqp,
        names["fc1_w"]: _pmajor(fc1_w).astype(BF),
        names["bf1"]: np.ascontiguousarray(
            np.asarray(fc1_b, np.float32).reshape(FT, P).T),
        names["fc2_w"]: _pmajor(fc2_w).astype(BF),
        names["f2bs"]: np.asarray(fc2_b).astype(BF).reshape(1, D),
    }
    in_maps = []
    for b in range(B):
        m = dict(common)
        m[names["x"]] = np.ascontiguousarray(
            x[b].reshape(TT, P, D).transpose(1, 0, 2))
        m[names["c"]] = np.ascontiguousarray(c[b].reshape(KT, P).T)
        in_maps.append(m)
    return in_maps


def kernel(x, c, ln1_w, ln1_b, ln2_w, ln2_b, ada_w, ada_b,
           qkv_w, qkv_b, proj_w, proj_b, fc1_w, fc1_b, fc2_w, fc2_b,
           _trace=False):
    nc, names = _get_compiled()
    in_maps = _prep_maps(names, x, c, ln1_w, ln1_b, ln2_w, ln2_b,
                         ada_w, ada_b, qkv_w, qkv_b, proj_w, proj_b,
                         fc1_w, fc1_b, fc2_w, fc2_b)
    res = bass_utils.run_bass_kernel_spmd(nc, in_maps, core_ids=list(range(B)),
                                          trace=_trace)
    out = np.stack([res.results[b][names["out"]] for b in range(B)])
    if _trace:
        _CACHE["last_result"] = res
    return out


# revision 29
# speedup vs baseline: 1.3661x; 1.3661x over previous
"""DiT block kernel for 8 Trainium2 NeuronCores.

Sharding: data-parallel over batch (B=8 -> one batch element per core).

Vs the original bf16 baseline:
  - scores / ada / fc1 / fc2 use 512-wide moving operands (baseline used
    256) — wide streams hide the PE weight-load latency.
  - q/k/attn stored fp8(e4m3): halves SBUF + eviction cost; scores and
    attn@V run with fp8 moving operands at bf16 rate; proj streams fp8
    per-column-quantized weights (dequant folded into the gate tile).
  - LN: token-major norm -> PE transpose -> modulate fused into ONE
    tensor_scalar per [128,512] pack at PSUM eviction.
  - fc1 PSUM evicts straight through ACT gelu (bias = per-partition AP).
  - softmax normalize: denominator row is PE-broadcast (ones-matmul) and
    inverted with reciprocal_approx_fast (NOT the 3.7us exact reciprocal).
  - phase-level pipelining: V-GEMM fills attn0's exp-wait; fc2 of chunk
    qc-1 fills attn(qc); gelu/exp never interleave (ACT table thrash).
"""

import sys

sys.path.insert(0, "/opt/trn_rl_repo")

import numpy as np
import ml_dtypes

import concourse.bacc as bacc
import concourse.tile as tile
from concourse import mybir
from concourse import bass_utils
from concourse.masks import make_identity

F32 = mybir.dt.float32
BF16 = mybir.dt.bfloat16
FP8 = mybir.dt.float8e4
BF = ml_dtypes.bfloat16
F8NP = ml_dtypes.float8_e4m3
OP = mybir.AluOpType
AF = mybir.ActivationFunctionType

B = 8
L = 1024
D = 1024
H = 16
HD = 64
FF = 4096
LN_EPS = 1e-5
P = 128
TT = L // P
KT = D // P
FT = FF // P
QC = 512
NQC = L // QC

S_QK = 32.0          # q/k fp8 scale
ESC = 0.125 / (S_QK * S_QK)

_CACHE = {}


def _build():
    nc = bacc.Bacc(None, target_bir_lowering=False)
    names = {}
    with tile.TileContext(nc) as tc, \
            tc.tile_pool(name="dram", bufs=1, space="DRAM") as dram, \
            tc.tile_pool(name="per", bufs=1) as per:

        x_d = dram.tile([P, TT, D], F32, kind="ExternalInput", name="x")
        c_d = dram.tile([P, KT], F32, kind="ExternalInput", name="c")
        lnT_d = dram.tile([P, 4 * KT], F32, kind="ExternalInput", name="lnT")
        adaw_d = dram.tile([P, KT, 6 * D], BF16, kind="ExternalInput",
                           name="ada_w")
        adab_d = dram.tile([1, 6 * D], BF16, kind="ExternalInput",
                           name="ada_b")
        qkvw_d = dram.tile([P, KT, 3 * D], BF16, kind="ExternalInput",
                           name="qkv_w")
        bqk_d = dram.tile([P, 16], F32, kind="ExternalInput", name="bqk")
        vbs_d = dram.tile([1, D], BF16, kind="ExternalInput", name="vbs")
        projw_d = dram.tile([P, KT, D], FP8, kind="ExternalInput",
                            name="proj_w")
        pbs_d = dram.tile([1, D], BF16, kind="ExternalInput", name="pbs")
        dqp_d = dram.tile([1, D], BF16, kind="ExternalInput", name="dqp")
        fc1w_d = dram.tile([P, KT, FF], BF16, kind="ExternalInput",
                           name="fc1_w")
        bf1_d = dram.tile([P, FT], F32, kind="ExternalInput", name="bf1")
        fc2w_d = dram.tile([P, FT, D], BF16, kind="ExternalInput",
                           name="fc2_w")
        f2bs_d = dram.tile([1, D], BF16, kind="ExternalInput", name="f2bs")
        out_d = dram.tile([L, D], F32, kind="ExternalOutput", name="out")
        for t, n in [(x_d, "x"), (c_d, "c"), (lnT_d, "lnT"),
                     (adaw_d, "ada_w"), (adab_d, "ada_b"),
                     (qkvw_d, "qkv_w"), (bqk_d, "bqk"), (vbs_d, "vbs"),
                     (projw_d, "proj_w"), (pbs_d, "pbs"), (dqp_d, "dqp"),
                     (fc1w_d, "fc1_w"), (bf1_d, "bf1"), (fc2w_d, "fc2_w"),
                     (f2bs_d, "f2bs"), (out_d, "out")]:
            names[n] = t.name
        out_v = out_d[:].rearrange("(t p) d -> p t d", p=P)

        x_sb = per.tile([P, TT, D], F32)
        for t in range(TT):
            nc.sync.dma_start(x_sb[:, t, :], x_d[:, t, :])
        ident = per.tile([P, P], BF16)
        make_identity(nc, ident[:])
        onesrow = per.tile([1, P], BF16)
        nc.vector.memset(onesrow[:], 1.0)
        eps_sb = per.tile([P, 1], F32)
        nc.vector.memset(eps_sb[:], LN_EPS)

        lnT = per.tile([P, 4 * KT], F32)
        nc.sync.dma_start(lnT[:], lnT_d[:])
        bqk = per.tile([P, 16], F32)
        nc.sync.dma_start(bqk[:], bqk_d[:])
        vbs = per.tile([1, D], BF16)
        nc.sync.dma_start(vbs[:], vbs_d[:])
        pbs = per.tile([1, D], BF16)
        nc.sync.dma_start(pbs[:], pbs_d[:])
        dqp_row = per.tile([1, D], BF16)
        nc.sync.dma_start(dqp_row[:], dqp_d[:])
        bf1 = per.tile([P, FT], F32)
        nc.sync.dma_start(bf1[:], bf1_d[:])
        f2bs = per.tile([1, D], BF16)
        nc.sync.dma_start(f2bs[:], f2bs_d[:])
        adab = per.tile([1, 6 * D], BF16)
        nc.sync.dma_start(adab[:], adab_d[:])
        projw = per.tile([P, KT, D], FP8)
        for k in range(KT):
            nc.sync.dma_start(projw[:, k, :], projw_d[:, k, :])

        eff1s = per.tile([P, KT], F32)
        eff1h = per.tile([P, KT], F32)
        eff2s = per.tile([P, KT], F32)
        eff2h = per.tile([P, KT], F32)
        g1bc = per.tile([P, D], BF16)
        g2bc = per.tile([P, D], BF16)

        q8 = per.tile([P, KT, L], FP8)
        k8 = per.tile([P, KT, L], FP8)
        v_sb = per.tile([P, TT, H, HD + 1], BF16)
        nc.vector.memset(v_sb[:, :, :, HD:HD + 1], 1.0)
        h2T = per.tile([P, KT, QC], BF16)
        geluT = per.tile([P, FT, QC], BF16)

        _work_cm = tc.tile_pool(name="work", bufs=2)
        work = _work_cm.__enter__()

        def ln_stats(x_aps):
            n = len(x_aps)
            mv = work.tile([P, n, 2], F32, tag=f"mv{n}", name=f"mv{n}")
            for i, x_ap in enumerate(x_aps):
                stats = work.tile([P, 2, 6], F32, tag="stats")
                for sg in range(2):
                    nc.vector.bn_stats(stats[:, sg, :],
                                       x_ap[:, sg * 512:(sg + 1) * 512])
                nc.vector.bn_aggr(mv[:, i, :], stats[:])
            rstd = work.tile([P, n], F32, tag=f"rstd{n}", name=f"rstd{n}")
            nc.scalar.activation(rstd[:], mv[:, :, 1], AF.Sqrt, bias=eps_sb[:])
            nc.vector.reciprocal(rstd[:], rstd[:])
            return mv, rstd

        mv1, rstd1 = ln_stats([x_sb[:, t, :] for t in range(TT)])

        # ---------------- ada (bf16, 512-wide moving weights) -----------
        with nc.named_scope("ada"), \
                tc.tile_pool(name="ada_sb", bufs=1) as asb, \
                tc.tile_pool(name="ada_stream", bufs=2) as ast, \
                tc.tile_pool(name="ps_ada", bufs=2, space="PSUM") as psa, \
                tc.tile_pool(name="ps_t6", bufs=1, space="PSUM") as pst6:
            c_sb = asb.tile([P, KT], F32)
            nc.sync.dma_start(c_sb[:], c_d[:])
            silu_b = asb.tile([P, KT], BF16)
            nc.scalar.activation(silu_b[:], c_sb[:], AF.Silu)
            ssgb = asb.tile([1, 6 * D], BF16, name="ssgb")
            for nch in range(12):
                aw = ast.tile([P, KT, 512], BF16, tag="aw")
                nc.sync.dma_start(aw[:],
                                  adaw_d[:, :, nch * 512:(nch + 1) * 512])
                pa = psa.tile([1, 512], F32, tag="ada")
                nc.tensor.matmul(pa[:], onesrow[:, 0:1],
                                 adab[:, nch * 512:(nch + 1) * 512],
                                 start=True, stop=False)
                for k in range(KT):
                    nc.tensor.matmul(pa[:], silu_b[:, k:k + 1], aw[:, k, :],
                                     start=False, stop=(k == KT - 1),
                                     skip_group_check=True)
                nc.vector.tensor_copy(
                    ssgb[:, nch * 512:(nch + 1) * 512], pa[:])
            # shift/scale rows -> feature-major cols; v' = sh1,sc1,sh2,sc2
            pt6 = pst6.tile([P, 32, 2], BF16)
            for vi, v in enumerate((0, 1, 3, 4)):
                for ch in range(KT):
                    i = ch * 4 + vi
                    nc.tensor.transpose(
                        pt6[:, i, 0:1],
                        ssgb[0:1, v * D + ch * P:v * D + (ch + 1) * P],
                        ident[0:1, 0:1])
            ssgT = asb.tile([P, KT, 4], F32, name="ssgT")
            nc.vector.tensor_copy(ssgT[:], pt6[:, :, 0])
            t0 = asb.tile([P, KT], F32, name="t0")
            t1 = asb.tile([P, KT], F32, name="t1")
            nc.vector.tensor_scalar_add(t0[:], ssgT[:, :, 1], 1.0)
            nc.vector.tensor_mul(eff1s[:], t0[:], lnT[:, 0:KT])
            nc.vector.tensor_mul(t1[:], t0[:], lnT[:, KT:2 * KT])
            nc.vector.tensor_add(eff1h[:], t1[:], ssgT[:, :, 0])
            nc.vector.tensor_scalar_add(t0[:], ssgT[:, :, 3], 1.0)
            nc.vector.tensor_mul(eff2s[:], t0[:], lnT[:, 2 * KT:3 * KT])
            nc.vector.tensor_mul(t1[:], t0[:], lnT[:, 3 * KT:4 * KT])
            nc.vector.tensor_add(eff2h[:], t1[:], ssgT[:, :, 2])
            g1row = asb.tile([1, D], BF16, name="g1row")
            nc.vector.tensor_mul(g1row[:], ssgb[:, 2 * D:3 * D], dqp_row[:])
            nc.gpsimd.partition_broadcast(g1bc[:], g1row[:])
            nc.gpsimd.partition_broadcast(g2bc[:], ssgb[:, 5 * D:6 * D])

        # ---------------- LN -> hT (transpose + fused modulate) ---------
        def build_hT(dst, eff_s, eff_h, mv, rstd, idx0, tg, toff, pstr):
            norms = []
            for s in range(4):
                t = toff + s
                nrm = work.tile([P, D], BF16, tag="nrm", bufs=4)
                i = idx0 + s
                nc.vector.tensor_scalar(out=nrm[:], in0=x_sb[:, t, :],
                                        scalar1=mv[:, i, 0:1],
                                        scalar2=rstd[:, i:i + 1],
                                        op0=OP.subtract, op1=OP.mult)
                norms.append(nrm)
            for k in range(KT):
                pt = pstr.tile([P, 4 * P], BF16, tag="tr")
                for s in range(4):
                    nc.tensor.transpose(pt[:, s * P:(s + 1) * P],
                                        norms[s][:, k * P:(k + 1) * P],
                                        ident[:])
                nc.vector.tensor_scalar(
                    out=dst[:, k, tg * 512:(tg + 1) * 512], in0=pt[:],
                    scalar1=eff_s[:, k:k + 1], scalar2=eff_h[:, k:k + 1],
                    op0=OP.mult, op1=OP.add)

        # ---------------- prologue: h1T + QKV (bf16, 512-wide) ----------
        pro_d = {}
        wv_half = []

        def open_prologue():
            pro_d["pro_cm"] = tc.tile_pool(name="pro", bufs=1)
            pro = pro_d["pro_cm"].__enter__()
            h1T = pro.tile([P, KT, L], BF16, name="h1T")
            with tc.tile_pool(name="ps_tr1", bufs=2, space="PSUM") as pstr1:
                for tg in range(2):
                    build_hT(h1T, eff1s, eff1h, mv1, rstd1, tg * 4, tg,
                             tg * 4, pstr1)
            pro_d["qs_cm"] = tc.tile_pool(name="qkv_stream", bufs=3)
            pro_d["qst"] = pro_d["qs_cm"].__enter__()
            pro_d["h1T"] = h1T

        def qk_chunk(qk, jj):
            fbase = D if qk == 0 else 0
            dst = k8 if qk == 0 else q8
            wj = pro_d["qst"].tile([P, KT, P], BF16, tag="wj", bufs=2,
                                   name="wjt")
            nc.sync.dma_start(
                wj[:], qkvw_d[:, :, fbase + jj * P:fbase + (jj + 1) * P])
            ci = (fbase + jj * P) // P
            for tg in range(2):
                pq = mm_tile(jj * 2 + tg, "pq")
                for k in range(KT):
                    nc.tensor.matmul(pq[:], wj[:, k, :],
                                     pro_d["h1T"][:, k,
                                                  tg * 512:(tg + 1) * 512],
                                     start=(k == 0), stop=(k == KT - 1))
                nc.vector.tensor_scalar(
                    out=dst[:, jj, tg * 512:(tg + 1) * 512], in0=pq[:],
                    scalar1=S_QK, scalar2=bqk[:, ci:ci + 1],
                    op0=OP.mult, op1=OP.add)

        def v_chunk(tt, fh):
            pv = mm_tile(tt * 2 + fh, "pv")
            nc.tensor.matmul(pv[:], onesrow[:, 0:P],
                             vbs[:, fh * 512:(fh + 1) * 512],
                             start=True, stop=False)
            for k in range(KT):
                nc.tensor.matmul(pv[:],
                                 pro_d["h1T"][:, k, tt * P:(tt + 1) * P],
                                 wv_half[fh][:, k, :],
                                 start=False, stop=(k == KT - 1),
                                 skip_group_check=True)
            nc.vector.tensor_copy(
                v_sb[:, tt, fh * 8:(fh + 1) * 8, 0:HD], pv[:])

        def load_wv(fh):
            wv = pro_d["qst"].tile([P, KT, 512], BF16, tag="wv", bufs=1,
                          name=f"wv{fh}")
            nc.sync.dma_start(wv[:],
                              qkvw_d[:, :, 2 * D + fh * 512:
                                     2 * D + (fh + 1) * 512])
            wv_half.append(wv)

        # ---------------- attention + MLP, phase-pipelined --------------
        fsd = {}

        def fs_tile(*a, **k):
            return fsd["fs"].tile(*a, **k)

        with tc.tile_pool(name="attn", bufs=2) as ap, \
                tc.tile_pool(name="aTp", bufs=1) as aTp, \
                tc.tile_pool(name="ps_s", bufs=2, space="PSUM") as pss, \
                tc.tile_pool(name="ps_av", bufs=1, space="PSUM") as psav, \
                tc.tile_pool(name="ps_m", bufs=1, space="PSUM") as psm:

            def mm_tile(i, nm):
                return psm.tile([P, 512], F32, tag=f"f2_{i % 2}", bufs=1,
                                name=nm)

            open_prologue()
            with nc.named_scope("qkv"):
                for jj in range(KT):
                    qk_chunk(0, jj)      # K
                for jj in range(KT):
                    qk_chunk(1, jj)      # Q
                for fh in range(2):
                    load_wv(fh)
                    for tt in range(TT):
                        v_chunk(tt, fh)

            def scores_exp(qc, j):
                q0 = qc * QC
                attA = ap.tile([P, TT, QC], FP8, tag="attA")
                attB = ap.tile([P, TT, QC], FP8, tag="attB")
                for m in range(TT):
                    psA = pss.tile([P, QC], F32, tag="sc", name="psA")
                    psB = pss.tile([P, QC], F32, tag="sc", name="psB")
                    nc.tensor.matmul(psA[:], k8[0:HD, j, m * P:(m + 1) * P],
                                     q8[0:HD, j, q0:q0 + QC],
                                     start=True, stop=True)
                    nc.tensor.matmul(psB[:], k8[HD:P, j, m * P:(m + 1) * P],
                                     q8[HD:P, j, q0:q0 + QC],
                                     start=True, stop=True,
                                     tile_position=(HD, 0))
                    nc.scalar.activation(attA[:, m, :], psA[:], AF.Exp,
                                         scale=ESC)
                    nc.scalar.activation(attB[:, m, :], psB[:], AF.Exp,
                                         scale=ESC)
                return attA, attB

            def av_norm(j, attA, attB, aT):
                for hh, att in ((0, attA), (1, attB)):
                    h = 2 * j + hh
                    pu = psav.tile([HD + 1, QC], F32, tag="pu")
                    for m in range(TT):
                        nc.tensor.matmul(pu[:], v_sb[:, m, h, :],
                                         att[:, m, :],
                                         start=(m == 0), stop=(m == TT - 1))
                    drow = work.tile([1, QC], BF16, tag="drow", bufs=1)
                    nc.vector.tensor_copy(drow[:], pu[HD:HD + 1, :])
                    rb = psav.tile([HD, QC], F32, tag="rb")
                    nc.tensor.matmul(rb[:], onesrow[:, 0:HD], drow[:],
                                     start=True, stop=True)
                    rec = work.tile([HD, QC], F32, tag="rec", bufs=1)
                    nc.vector.reciprocal_approx_fast(rec[:], rb[:])
                    nc.vector.tensor_tensor(
                        out=aT[hh * 64:(hh + 1) * 64, j, :],
                        in0=pu[0:HD, :], in1=rec[:], op=OP.mult)

            def proj_block(qc, aT, s):
                t_global = qc * 4 + s
                for fh in range(2):
                    pp = mm_tile(s * 2 + fh, "pp")
                    nc.tensor.matmul(pp[:], onesrow[:, 0:P],
                                     pbs[:, fh * 512:(fh + 1) * 512],
                                     start=True, stop=False)
                    for k in range(KT):
                        nc.tensor.matmul(
                            pp[:], aT[:, k, s * P:(s + 1) * P],
                            projw[:, k, fh * 512:(fh + 1) * 512],
                            start=False, stop=(k == KT - 1),
                            skip_group_check=True)
                    tmp = work.tile([P, 512], BF16, tag="ptmp", bufs=1)
                    nc.vector.tensor_tensor(
                        out=tmp[:], in0=pp[:],
                        in1=g1bc[:, fh * 512:(fh + 1) * 512], op=OP.mult)
                    nc.vector.tensor_add(
                        x_sb[:, t_global, fh * 512:(fh + 1) * 512],
                        x_sb[:, t_global, fh * 512:(fh + 1) * 512],
                        tmp[:])

            def fc1_gelu(qc):
                for fc in range(FT):
                    w1 = fs_tile([P, KT, P], BF16, tag="w1", bufs=3,
                                 name="w1t")
                    nc.sync.dma_start(w1[:],
                                      fc1w_d[:, :, fc * P:(fc + 1) * P])
                    pf = mm_tile(fc, "pf")
                    for k in range(KT):
                        nc.tensor.matmul(pf[:], w1[:, k, :], h2T[:, k, :],
                                         start=(k == 0), stop=(k == KT - 1))
                    nc.scalar.activation(geluT[:, fc, :], pf[:],
                                         AF.Gelu_apprx_tanh,
                                         bias=bf1[:, fc:fc + 1])

            def fc2_blocks(qc):
                # s-pairs share the streamed w2 chunk; 2 psum banks live
                for sp in range(2):
                    for fp in range(2):
                        def blk(sp=sp, fp=fp, qc=qc):
                            ps2 = [mm_tile(0, "pf2a"), mm_tile(1, "pf2b")]
                            for s in range(2):
                                nc.tensor.matmul(
                                    ps2[s][:], onesrow[:, 0:P],
                                    f2bs[:, fp * 512:(fp + 1) * 512],
                                    start=True, stop=False)
                            for ft in range(FT):
                                w2 = fs_tile([P, 512], BF16, tag="w2",
                                             bufs=3, name="w2t")
                                nc.sync.dma_start(
                                    w2[:], fc2w_d[:, ft, fp * 512:
                                                  (fp + 1) * 512])
                                for s in range(2):
                                    tok = sp * 2 + s
                                    nc.tensor.matmul(
                                        ps2[s][:],
                                        geluT[:, ft,
                                              tok * P:(tok + 1) * P],
                                        w2[:],
                                        start=False, stop=(ft == FT - 1),
                                        skip_group_check=True)
                            for s in range(2):
                                t_global = qc * 4 + sp * 2 + s
                                tmp = work.tile([P, 512], BF16, tag="ftmp",
                                                bufs=1)
                                nc.vector.tensor_tensor(
                                    out=tmp[:], in0=ps2[s][:],
                                    in1=g2bc[:, fp * 512:(fp + 1) * 512],
                                    op=OP.mult)
                                oth = work.tile([P, 512], F32, tag="ot",
                                                bufs=2,
                                                name=f"ot{qc}{fp}{sp}{s}")
                                nc.vector.tensor_add(
                                    oth[:],
                                    x_sb[:, t_global,
                                         fp * 512:(fp + 1) * 512],
                                    tmp[:])
                                nc.sync.dma_start(
                                    out_v[:, t_global,
                                          fp * 512:(fp + 1) * 512],
                                    oth[:])
                        yield blk

            closed = {}

            def run_qc(qc, pending):
                pend_i = 0
                with nc.named_scope(f"attn{qc}"):
                    aT = aTp.tile([P, KT, QC], BF16, tag="aT",
                                  name=f"aT_{qc}")
                    atts = []
                    for j in range(KT):
                        atts.append(scores_exp(qc, j))
                        for _ in range(2):
                            if pend_i < len(pending):
                                pending[pend_i]()
                                pend_i += 1
                        if j >= 1:
                            av_norm(j - 1, *atts[j - 1], aT)
                    av_norm(KT - 1, *atts[KT - 1], aT)
                if qc == 0 and not closed:
                    # h1T / V weights are dead: free prologue SBUF
                    pro_d["qs_cm"].__exit__(None, None, None)
                    pro_d["pro_cm"].__exit__(None, None, None)
                    fsd["cm"] = tc.tile_pool(name="fc_stream", bufs=2)
                    fsd["fs"] = fsd["cm"].__enter__()
                    closed["done"] = True
                with nc.named_scope(f"proj{qc}"):
                    for s in range(4):
                        proj_block(qc, aT, s)
                with nc.named_scope(f"ln2_{qc}"):
                    mv2, rstd2 = ln_stats(
                        [x_sb[:, qc * 4 + s, :] for s in range(4)])
                    with tc.tile_pool(name="ps_tr2", bufs=2,
                                      space="PSUM") as pstr2:
                        build_hT(h2T, eff2s, eff2h, mv2, rstd2, 0, 0,
                                 qc * 4, pstr2)
                with nc.named_scope(f"fc1_{qc}"):
                    fc1_gelu(qc)
                    while pend_i < len(pending):
                        pending[pend_i]()
                        pend_i += 1
                return list(fc2_blocks(qc))

            pending = run_qc(0, [])
            pending = run_qc(1, pending)
            with nc.named_scope("mlp_tail"):
                for blk in pending:
                    blk()
            fsd["cm"].__exit__(None, None, None)

        _work_cm.__exit__(None, None, None)

    nc.compile()
    return nc, names


def _get_compiled():
    if "nc" not in _CACHE:
        _CACHE["nc"], _CACHE["names"] = _build()
    return _CACHE["nc"], _CACHE["names"]


def _q8col(w):
    w = np.asarray(w, np.float32)
    am = np.abs(w).max(axis=0, keepdims=True)
    s = np.where(am > 0, 224.0 / np.maximum(am, 1e-30), 1.0)
    w8 = np.clip(w * s, -240, 240).astype(F8NP)
    return w8, s[0]


def _pmajor(w):
    w = np.asarray(w)
    kp, n = w.shape
    return np.ascontiguousarray(w.reshape(kp // P, P, n).transpose(1, 0, 2))


def _prep_maps(names, x, c, ln1_w, ln1_b, ln2_w, ln2_b, ada_w, ada_b,
               qkv_w, qkv_b, proj_w, proj_b, fc1_w, fc1_b, fc2_w, fc2_b):
    x = np.asarray(x, np.float32)
    c = np.asarray(c, np.float32)
    proj8, s_proj = _q8col(proj_w)
    qkv_b = np.asarray(qkv_b, np.float32)

    def tcols(v):
        return np.asarray(v, np.float32).reshape(KT, P).T
    lnT = np.concatenate([tcols(ln1_w), tcols(ln1_b),
                          tcols(ln2_w), tcols(ln2_b)], axis=1)

    common = {
        names["lnT"]: np.ascontiguousarray(lnT, np.float32),
        names["ada_w"]: _pmajor(ada_w).astype(BF),
        names["ada_b"]: np.asarray(ada_b).astype(BF).reshape(1, -1),
        names["qkv_w"]: _pmajor(qkv_w).astype(BF),
        names["bqk"]: np.ascontiguousarray(
            (qkv_b[0:2 * D] * S_QK).reshape(16, P).T.astype(np.float32)),
        names["vbs"]: qkv_b[2 * D:].astype(BF).reshape(1, D),
        names["proj_w"]: _pmajor(proj8),
        names["pbs"]: (np.asarray(proj_b, np.float32) * s_proj)
        .astype(BF).reshape(1, D),
        names["dqp"]: (1.0 / s_proj).astype(BF).reshape(1, D),
        names["fc1_w"]: _pmajor(fc1_w).astype(BF),
        names["bf1"]: np.ascontiguousarray(
            np.asarray(fc1_b, np.float32).reshape(FT, P).T),
        names["fc2_w"]: _pmajor(fc2_w).astype(BF),
        names["f2bs"]: np.asarray(fc2_b).astype(BF).reshape(1, D),
    }
    in_maps = []
    for b in range(B):
        m = dict(common)
        m[names["x"]] = np.ascontiguousarray(
            x[b].reshape(TT, P, D).transpose(1, 0, 2))
        m[names["c"]] = np.ascontiguousarray(c[b].reshape(KT, P).T)
        in_maps.append(m)
    return in_maps


def kernel(x, c, ln1_w, ln1_b, ln2_w, ln2_b, ada_w, ada_b,
           qkv_w, qkv_b, proj_w, proj_b, fc1_w, fc1_b, fc2_w, fc2_b,
           _trace=False):
    nc, names = _get_compiled()
    in_maps = _prep_maps(names, x, c, ln1_w, ln1_b, ln2_w, ln2_b,
                         ada_w, ada_b, qkv_w, qkv_b, proj_w, proj_b,
                         fc1_w, fc1_b, fc2_w, fc2_b)
    res = bass_utils.run_bass_kernel_spmd(nc, in_maps, core_ids=list(range(B)),
                                          trace=_trace)
    out = np.stack([res.results[b][names["out"]] for b in range(B)])
    if _trace:
        _CACHE["last_result"] = res
    return out
